# revision 18
# baseline (speedup 1.0000x reference)
"""Causal grouped Conv1d on 8 Trainium2 NeuronCores.

Problem: x [B=4, L=4096, D=2048] f32, w [K=4, D/G=256, D=2048] f32, G=8 groups.
out[b, l, o] = sum_{k, i} x[b, l-3+k, g(o)*256 + i] * w[k, i, o]   (causal pad 3)

Sharding: data-parallel over (B, L/2) -> 8 shards of 2048 tokens each.
Each core gets its token rows plus a 3-row left halo (zeros at batch start).

Per-core kernel (Bass/Tile):
  - PE-transpose x tiles [128 tok, 128 cin] -> xT [128 cin, 131 tok] (incl halo)
  - grouped matmuls: psum[128 tok, 256 och] += xT[:, k:k+128].T @ w[k, cin, och]
    accumulating over k in 0..3 and the group's two 128-cin chunks.
"""

import numpy as np

import concourse.bass as bass
import concourse.mybir as mybir
import concourse.tile as tile
from concourse import bacc, masks
from concourse.bass_utils import run_bass_kernel_spmd

B, L, D, K, G = 4, 4096, 2048, 4, 8
CG = D // G          # 256 channels per group
NCORES = 8
TOK = (B * L) // NCORES   # 2048 tokens per core
TT = 128                  # token tile
NT = TOK // TT            # 16 token tiles
NCHUNK = D // 128         # 16 cin chunks of 128
PAD = K - 1               # 3 (causal left pad)

DT = mybir.dt.float32     # storage/transpose dtype
F32 = mybir.dt.float32
F32R = mybir.dt.float32r
MM_F32R = True            # run conv matmuls in float32r


TB = 512                  # token block for the matmul moving dim
NB = TOK // TB            # 4 token blocks per core


def _emit(tc, nc, xs, wt, y):
    """y is [D, TOK] (transposed); host un-transposes."""
    import contextlib
    ctx = contextlib.ExitStack()
    mmdt = F32R if MM_F32R else DT
    with ctx:
        constp = ctx.enter_context(tc.tile_pool(name="constp", bufs=1))
        wp = ctx.enter_context(tc.tile_pool(name="wp", bufs=1))
        xinp = ctx.enter_context(tc.tile_pool(name="xinp", bufs=9))
        halop = ctx.enter_context(tc.tile_pool(name="halop", bufs=2))
        xtp = ctx.enter_context(tc.tile_pool(name="xtp", bufs=6))
        outp = ctx.enter_context(tc.tile_pool(name="outp", bufs=4))
        pm = ctx.enter_context(tc.tile_pool(name="pm", bufs=2, space="PSUM"))
        ph = ctx.enter_context(tc.tile_pool(name="ph", bufs=2, space="PSUM"))
        po = ctx.enter_context(tc.tile_pool(name="po", bufs=4, space="PSUM"))

        ident = constp.tile([128, 128], DT)
        masks.make_identity(nc, ident[:])

        def issue_x_dmas(t0):
            xh = halop.tile([PAD, D], DT, name="xh")
            nc.sync.dma_start(xh[:], xs[t0: t0 + PAD, :])
            xms = []
            for i in range(TB // TT):
                xm = xinp.tile([TT, D], DT, name="xm")
                nc.sync.dma_start(
                    xm[:], xs[PAD + t0 + i * TT: PAD + t0 + (i + 1) * TT, :]
                )
                xms.append(xm)
            return xms, xh

        # Prefetch block 0's x before the weight loads so the SP HWDGE ring
        # serves the first transposes immediately.
        x0 = issue_x_dmas(0)

        # Weights resident in SBUF: w_sb[(k, j)] = [128 cin, 2048 och].
        # Loaded on the ACT HWDGE ring (doesn't block x loads), quartered by
        # output-channel so early groups' weights land first.
        wsb = {}
        for k in range(K):
            for j in range(2):
                wsb[(k, j)] = wp.tile([128, D], mmdt, name=f"w_{k}_{j}")
        QW = D // 4
        for q in range(4):
            for k in range(K):
                for j in range(2):
                    nc.scalar.dma_start(
                        wsb[(k, j)][:, q * QW:(q + 1) * QW],
                        wt[k, j * 128:(j + 1) * 128, q * QW:(q + 1) * QW],
                    )

        for t in range(NB):
            t0 = t * TB
            xms, xh = x0 if t == 0 else issue_x_dmas(t0)

            def make_xt(c):
                pmt = pm.tile([128, TB], DT, name="pmt")
                for i in range(TB // TT):
                    nc.tensor.transpose(
                        pmt[:, i * TT:(i + 1) * TT],
                        xms[i][:, c * 128:(c + 1) * 128],
                        ident[:],
                    )
                pht = ph.tile([128, PAD], DT, name="pht")
                nc.tensor.transpose(
                    pht[:], xh[:, c * 128:(c + 1) * 128], ident[0:PAD, 0:PAD]
                )
                xt_t = xtp.tile([128, TB + PAD], mmdt, name="xt_t")
                nc.vector.tensor_copy(xt_t[:, PAD:], pmt[:])
                nc.vector.tensor_copy(xt_t[:, 0:PAD], pht[:])
                return xt_t

            for g in range(G):
                xt_pair = [make_xt(2 * g + 0), make_xt(2 * g + 1)]
                for cc in (2 * g, 2 * g + 1):
                    pot = po.tile([128, TB], F32, name="pot")
                    first = True
                    for j in range(2):
                        xt_t = xt_pair[j]
                        for k in range(K):
                            nc.tensor.matmul(
                                pot[:],
                                wsb[(k, j)][:, cc * 128:(cc + 1) * 128],
                                xt_t[:, k:k + TB],
                                start=first,
                                stop=(j == 1 and k == K - 1),
                            )
                            first = False
                    ot = outp.tile([128, TB], F32, name="ot")
                    nc.vector.tensor_copy(ot[:], pot[:])
                    nc.sync.dma_start(
                        y[cc * 128:(cc + 1) * 128, t0:t0 + TB], ot[:]
                    )


_NC_CACHE = None


def build_nc():
    global _NC_CACHE
    if _NC_CACHE is not None:
        return _NC_CACHE
    nc = bacc.Bacc(
        "TRN2", target_bir_lowering=False, debug=False, num_devices=NCORES
    )
    xs = nc.dram_tensor("xs", [TOK + PAD, D], DT, kind="ExternalInput").ap()
    wt = nc.dram_tensor(
        "wt", [K, CG, D], F32R if MM_F32R else DT, kind="ExternalInput"
    ).ap()
    y = nc.dram_tensor("y", [D, TOK], F32, kind="ExternalOutput").ap()
    with tile.TileContext(nc) as tc:
        _emit(tc, nc, xs, wt, y)
    nc.compile()
    _NC_CACHE = nc
    return nc


def make_in_maps(x, w):
    """Shard full x [B, L, D] into 8 per-core inputs with causal halo rows."""
    x = np.ascontiguousarray(x, dtype=np.float32)
    w = np.ascontiguousarray(w, dtype=np.float32)
    halves = L // (NCORES // B)  # 2048
    in_maps = []
    for core in range(NCORES):
        b, h = divmod(core, NCORES // B)
        lo = h * halves
        shard = np.zeros((TOK + PAD, D), dtype=np.float32)
        if lo == 0:
            shard[PAD:] = x[b, lo:lo + TOK]
        else:
            shard[:] = x[b, lo - PAD:lo + TOK]
        in_maps.append({"xs": shard, "wt": w})
    return in_maps


def run(x, w, trace=False, **kw):
    nc = build_nc()
    res = run_bass_kernel_spmd(
        nc, make_in_maps(x, w), core_ids=list(range(NCORES)), trace=trace, **kw
    )
    halves = L // (NCORES // B)
    out = np.empty((B, L, D), dtype=np.float32)
    for core in range(NCORES):
        b, h = divmod(core, NCORES // B)
        out[b, h * halves:(h + 1) * halves] = res.results[core]["y"].T
    return out, res


def kernel(x, w):
    out, _ = run(x, w, trace=False)
    return out


# revision 21
# speedup vs baseline: 1.0068x; 1.0068x over previous
"""Causal grouped Conv1d on 8 Trainium2 NeuronCores.

Problem: x [B=4, L=4096, D=2048] f32, w [K=4, D/G=256, D=2048] f32, G=8 groups.
out[b, l, o] = sum_{k, i} x[b, l-3+k, g(o)*256 + i] * w[k, i, o]   (causal pad 3)

Sharding: data-parallel over (B, L/2) -> 8 shards of 2048 tokens each.
Each core gets its token rows plus a 3-row left halo (zeros at batch start).

Per-core kernel (Bass/Tile):
  - PE-transpose x tiles [128 tok, 128 cin] -> xT [128 cin, 131 tok] (incl halo)
  - grouped matmuls: psum[128 tok, 256 och] += xT[:, k:k+128].T @ w[k, cin, och]
    accumulating over k in 0..3 and the group's two 128-cin chunks.
"""

import numpy as np

import concourse.bass as bass
import concourse.mybir as mybir
import concourse.tile as tile
from concourse import bacc, masks
from concourse.bass_utils import run_bass_kernel_spmd

B, L, D, K, G = 4, 4096, 2048, 4, 8
CG = D // G          # 256 channels per group
NCORES = 8
TOK = (B * L) // NCORES   # 2048 tokens per core
TT = 128                  # token tile
NT = TOK // TT            # 16 token tiles
NCHUNK = D // 128         # 16 cin chunks of 128
PAD = K - 1               # 3 (causal left pad)

DT = mybir.dt.float32     # storage/transpose dtype
F32 = mybir.dt.float32
F32R = mybir.dt.float32r
MM_F32R = True            # run conv matmuls in float32r


TB = 512                  # token block for the matmul moving dim
NB = TOK // TB            # 4 token blocks per core


def _emit(tc, nc, xs, wt, y):
    """y is [D, TOK] (transposed); host un-transposes."""
    import contextlib
    ctx = contextlib.ExitStack()
    mmdt = F32R if MM_F32R else DT
    with ctx:
        constp = ctx.enter_context(tc.tile_pool(name="constp", bufs=1))
        wp = ctx.enter_context(tc.tile_pool(name="wp", bufs=1))
        xinp = ctx.enter_context(tc.tile_pool(name="xinp", bufs=7))
        halop = ctx.enter_context(tc.tile_pool(name="halop", bufs=2))
        xtp = ctx.enter_context(tc.tile_pool(name="xtp", bufs=18))
        outp = ctx.enter_context(tc.tile_pool(name="outp", bufs=4))
        pm = ctx.enter_context(tc.tile_pool(name="pm", bufs=2, space="PSUM"))
        ph = ctx.enter_context(tc.tile_pool(name="ph", bufs=2, space="PSUM"))
        po = ctx.enter_context(tc.tile_pool(name="po", bufs=4, space="PSUM"))

        ident = constp.tile([128, 128], DT)
        masks.make_identity(nc, ident[:])

        def issue_x_dmas(t0):
            xh = halop.tile([PAD, D], DT, name="xh")
            nc.sync.dma_start(xh[:], xs[t0: t0 + PAD, :])
            xms = []
            for i in range(TB // TT):
                xm = xinp.tile([TT, D], DT, name="xm")
                nc.sync.dma_start(
                    xm[:], xs[PAD + t0 + i * TT: PAD + t0 + (i + 1) * TT, :]
                )
                xms.append(xm)
            return xms, xh

        # Prefetch block 0's x before the weight loads so the SP HWDGE ring
        # serves the first transposes immediately.
        x0 = issue_x_dmas(0)

        # Weights resident in SBUF: w_sb[(k, j)] = [128 cin, 2048 och].
        # Loaded on the ACT HWDGE ring (doesn't block x loads), quartered by
        # output-channel so early groups' weights land first.
        wsb = {}
        for k in range(K):
            for j in range(2):
                wsb[(k, j)] = wp.tile([128, D], mmdt, name=f"w_{k}_{j}")
        for g in range(G):
            for k in range(K):
                for j in range(2):
                    nc.scalar.dma_start(
                        wsb[(k, j)][:, g * CG:(g + 1) * CG],
                        wt[k, j * 128:(j + 1) * 128, g * CG:(g + 1) * CG],
                    )

        for t in range(NB):
            t0 = t * TB
            xms, xh = x0 if t == 0 else issue_x_dmas(t0)

            def make_xt(c):
                pmt = pm.tile([128, TB], DT, name="pmt")
                for i in range(TB // TT):
                    nc.tensor.transpose(
                        pmt[:, i * TT:(i + 1) * TT],
                        xms[i][:, c * 128:(c + 1) * 128],
                        ident[:],
                    )
                pht = ph.tile([128, PAD], DT, name="pht")
                nc.tensor.transpose(
                    pht[:], xh[:, c * 128:(c + 1) * 128], ident[0:PAD, 0:PAD]
                )
                xt_t = xtp.tile([128, TB + PAD], mmdt, name="xt_t")
                nc.vector.tensor_copy(xt_t[:, PAD:], pmt[:])
                nc.vector.tensor_copy(xt_t[:, 0:PAD], pht[:])
                return xt_t

            xts = [make_xt(c) for c in range(NCHUNK)]
            for g in range(G):
                xt_pair = [xts[2 * g + 0], xts[2 * g + 1]]
                for cc in (2 * g, 2 * g + 1):
                    pot = po.tile([128, TB], F32, name="pot")
                    first = True
                    for j in range(2):
                        xt_t = xt_pair[j]
                        for k in range(K):
                            nc.tensor.matmul(
                                pot[:],
                                wsb[(k, j)][:, cc * 128:(cc + 1) * 128],
                                xt_t[:, k:k + TB],
                                start=first,
                                stop=(j == 1 and k == K - 1),
                            )
                            first = False
                    ot = outp.tile([128, TB], F32, name="ot")
                    nc.vector.tensor_copy(ot[:], pot[:])
                    nc.sync.dma_start(
                        y[cc * 128:(cc + 1) * 128, t0:t0 + TB], ot[:]
                    )


_NC_CACHE = None


def build_nc():
    global _NC_CACHE
    if _NC_CACHE is not None:
        return _NC_CACHE
    nc = bacc.Bacc(
        "TRN2", target_bir_lowering=False, debug=False, num_devices=NCORES
    )
    xs = nc.dram_tensor("xs", [TOK + PAD, D], DT, kind="ExternalInput").ap()
    wt = nc.dram_tensor(
        "wt", [K, CG, D], F32R if MM_F32R else DT, kind="ExternalInput"
    ).ap()
    y = nc.dram_tensor("y", [D, TOK], F32, kind="ExternalOutput").ap()
    with tile.TileContext(nc) as tc:
        _emit(tc, nc, xs, wt, y)
    nc.compile()
    _NC_CACHE = nc
    return nc


def make_in_maps(x, w):
    """Shard full x [B, L, D] into 8 per-core inputs with causal halo rows."""
    x = np.ascontiguousarray(x, dtype=np.float32)
    w = np.ascontiguousarray(w, dtype=np.float32)
    halves = L // (NCORES // B)  # 2048
    in_maps = []
    for core in range(NCORES):
        b, h = divmod(core, NCORES // B)
        lo = h * halves
        shard = np.zeros((TOK + PAD, D), dtype=np.float32)
        if lo == 0:
            shard[PAD:] = x[b, lo:lo + TOK]
        else:
            shard[:] = x[b, lo - PAD:lo + TOK]
        in_maps.append({"xs": shard, "wt": w})
    return in_maps


def run(x, w, trace=False, **kw):
    nc = build_nc()
    res = run_bass_kernel_spmd(
        nc, make_in_maps(x, w), core_ids=list(range(NCORES)), trace=trace, **kw
    )
    halves = L // (NCORES // B)
    out = np.empty((B, L, D), dtype=np.float32)
    for core in range(NCORES):
        b, h = divmod(core, NCORES // B)
        out[b, h * halves:(h + 1) * halves] = res.results[core]["y"].T
    return out, res


def kernel(x, w):
    out, _ = run(x, w, trace=False)
    return out


# revision 22
# speedup vs baseline: 1.0270x; 1.0201x over previous
"""Causal grouped Conv1d on 8 Trainium2 NeuronCores.

Problem: x [B=4, L=4096, D=2048] f32, w [K=4, D/G=256, D=2048] f32, G=8 groups.
out[b, l, o] = sum_{k, i} x[b, l-3+k, g(o)*256 + i] * w[k, i, o]   (causal pad 3)

Sharding: data-parallel over (B, L/2) -> 8 shards of 2048 tokens each.
Each core gets its token rows plus a 4-row left halo (zeros at batch start;
only the last 3 halo rows are used).

Per-core kernel (Bass/Tile, "Form B"):
  - PE-transpose x into xT tiles [128 cin, 515 tok] per 512-token block
  - conv matmuls (float32r): psum[128 och, 512 tok] += w[k,cin,och].T @
    xT[:, k:k+512], accumulating over k in 0..3 and the group's 2 cin chunks
  - output written och-major [D, TOK] to HBM; host transposes back.
"""

import numpy as np

import concourse.bass as bass
import concourse.mybir as mybir
import concourse.tile as tile
from concourse import bacc, masks
from concourse.bass_utils import run_bass_kernel_spmd

B, L, D, K, G = 4, 4096, 2048, 4, 8
CG = D // G          # 256 channels per group
NCORES = 8
TOK = (B * L) // NCORES   # 2048 tokens per core
TT = 128                  # row tile for DMA/transpose
NCHUNK = D // 128         # 16 cin chunks of 128
PAD = K - 1               # 3 (causal left pad)
HPAD = 4                  # halo rows fetched (fp32r transpose needs even cols)

F32 = mybir.dt.float32
F32R = mybir.dt.float32r
MM_F32R = True            # conv matmuls + transposes in float32r

TB = 512                  # token block for the matmul moving dim
NB = TOK // TB            # 4 token blocks per core


def _emit(tc, nc, xs, wt, y):
    """y is [D, TOK] (transposed); host un-transposes."""
    import contextlib
    ctx = contextlib.ExitStack()
    mmdt = F32R if MM_F32R else F32
    with ctx:
        constp = ctx.enter_context(tc.tile_pool(name="constp", bufs=1))
        wp = ctx.enter_context(tc.tile_pool(name="wp", bufs=1))
        xinp = ctx.enter_context(tc.tile_pool(name="xinp", bufs=7))
        halop = ctx.enter_context(tc.tile_pool(name="halop", bufs=2))
        xtp = ctx.enter_context(tc.tile_pool(name="xtp", bufs=18))
        outp = ctx.enter_context(tc.tile_pool(name="outp", bufs=4))
        pm = ctx.enter_context(tc.tile_pool(name="pm", bufs=2, space="PSUM"))
        ph = ctx.enter_context(tc.tile_pool(name="ph", bufs=2, space="PSUM"))
        po = ctx.enter_context(tc.tile_pool(name="po", bufs=4, space="PSUM"))

        identf = constp.tile([128, 128], F32)
        masks.make_identity(nc, identf[:])
        if MM_F32R:
            ident = constp.tile([128, 128], F32R)
            nc.vector.tensor_copy(ident[:], identf[:])
        else:
            ident = identf

        def issue_x_dmas(t0):
            """x rows [t0-4, t0+TB) of this core's token range.

            xs row r = token r - HPAD."""
            xh = halop.tile([HPAD, D], mmdt, name="xh")
            nc.sync.dma_start(xh[:], xs[t0: t0 + HPAD, :])
            xms = []
            for i in range(TB // TT):
                xm = xinp.tile([TT, D], mmdt, name="xm")
                nc.sync.dma_start(
                    xm[:], xs[HPAD + t0 + i * TT: HPAD + t0 + (i + 1) * TT, :]
                )
                xms.append(xm)
            return xms, xh

        # Prefetch block 0's x before the weight loads so the SP HWDGE ring
        # serves the first transposes immediately.
        x0 = issue_x_dmas(0)

        # Weights resident in SBUF: w_sb[(k, j)] = [128 cin, 2048 och].
        # Loaded on the ACT HWDGE ring (doesn't block x loads), group-major so
        # early groups' weights land first.
        wsb = {}
        for k in range(K):
            for j in range(2):
                wsb[(k, j)] = wp.tile([128, D], mmdt, name=f"w_{k}_{j}")
        for g in range(G):
            for k in range(K):
                for j in range(2):
                    nc.scalar.dma_start(
                        wsb[(k, j)][:, g * CG:(g + 1) * CG],
                        wt[k, j * 128:(j + 1) * 128, g * CG:(g + 1) * CG],
                    )

        for t in range(NB):
            t0 = t * TB
            xms, xh = x0 if t == 0 else issue_x_dmas(t0)

            def make_xt(c):
                # pmt cols = tokens [t0, t0+TB)
                pmt = pm.tile([128, TB], mmdt, name="pmt")
                for i in range(TB // TT):
                    nc.tensor.transpose(
                        pmt[:, i * TT:(i + 1) * TT],
                        xms[i][:, c * 128:(c + 1) * 128],
                        ident[:],
                    )
                # pht cols = tokens [t0-4, t0)
                pht = ph.tile([128, HPAD], mmdt, name="pht")
                nc.tensor.transpose(
                    pht[:], xh[:, c * 128:(c + 1) * 128], ident[0:HPAD, 0:HPAD]
                )
                # xt cols = tokens [t0-3, t0+TB)
                xt_t = xtp.tile([128, TB + PAD], mmdt, name="xt_t")
                nc.vector.tensor_copy(xt_t[:, PAD:], pmt[:])
                nc.vector.tensor_copy(xt_t[:, 0:PAD], pht[:, HPAD - PAD:])
                return xt_t

            xts = [make_xt(c) for c in range(NCHUNK)]
            for g in range(G):
                xt_pair = [xts[2 * g + 0], xts[2 * g + 1]]
                for cc in (2 * g, 2 * g + 1):
                    pot = po.tile([128, TB], F32, name="pot")
                    first = True
                    for j in range(2):
                        xt_t = xt_pair[j]
                        for k in range(K):
                            nc.tensor.matmul(
                                pot[:],
                                wsb[(k, j)][:, cc * 128:(cc + 1) * 128],
                                xt_t[:, k:k + TB],
                                start=first,
                                stop=(j == 1 and k == K - 1),
                            )
                            first = False
                    ot = outp.tile([128, TB], F32, name="ot")
                    nc.vector.tensor_copy(ot[:], pot[:])
                    nc.sync.dma_start(
                        y[cc * 128:(cc + 1) * 128, t0:t0 + TB], ot[:]
                    )


_NC_CACHE = None


def build_nc():
    global _NC_CACHE
    if _NC_CACHE is not None:
        return _NC_CACHE
    mmdt = F32R if MM_F32R else F32
    nc = bacc.Bacc(
        "TRN2", target_bir_lowering=False, debug=False, num_devices=NCORES
    )
    xs = nc.dram_tensor("xs", [TOK + HPAD, D], mmdt, kind="ExternalInput").ap()
    wt = nc.dram_tensor("wt", [K, CG, D], mmdt, kind="ExternalInput").ap()
    y = nc.dram_tensor("y", [D, TOK], F32, kind="ExternalOutput").ap()
    with tile.TileContext(nc) as tc:
        _emit(tc, nc, xs, wt, y)
    nc.compile()
    _NC_CACHE = nc
    return nc


def make_in_maps(x, w):
    """Shard full x [B, L, D] into 8 per-core inputs with causal halo rows."""
    x = np.ascontiguousarray(x, dtype=np.float32)
    w = np.ascontiguousarray(w, dtype=np.float32)
    halves = L // (NCORES // B)  # 2048
    in_maps = []
    for core in range(NCORES):
        b, h = divmod(core, NCORES // B)
        lo = h * halves
        shard = np.zeros((TOK + HPAD, D), dtype=np.float32)
        if lo == 0:
            shard[HPAD:] = x[b, lo:lo + TOK]
        else:
            shard[HPAD - min(lo, HPAD):] = x[b, lo - min(lo, HPAD):lo + TOK]
        in_maps.append({"xs": shard, "wt": w})
    return in_maps


def run(x, w, trace=False, **kw):
    nc = build_nc()
    res = run_bass_kernel_spmd(
        nc, make_in_maps(x, w), core_ids=list(range(NCORES)), trace=trace, **kw
    )
    halves = L // (NCORES // B)
    out = np.empty((B, L, D), dtype=np.float32)
    for core in range(NCORES):
        b, h = divmod(core, NCORES // B)
        out[b, h * halves:(h + 1) * halves] = res.results[core]["y"].T
    return out, res


def kernel(x, w):
    out, _ = run(x, w, trace=False)
    return out


# revision 23
# speedup vs baseline: 1.0425x; 1.0151x over previous
"""Causal grouped Conv1d on 8 Trainium2 NeuronCores.

Problem: x [B=4, L=4096, D=2048] f32, w [K=4, D/G=256, D=2048] f32, G=8 groups.
out[b, l, o] = sum_{k, i} x[b, l-3+k, g(o)*256 + i] * w[k, i, o]   (causal pad 3)

Sharding: data-parallel over (B, L/2) -> 8 shards of 2048 tokens each.
Each core gets its token rows plus a 4-row left halo (zeros at batch start;
only the last 3 halo rows are used).

Per-core kernel (Bass/Tile, "Form B"):
  - PE-transpose x into xT tiles [128 cin, 515 tok] per 512-token block
  - conv matmuls (float32r): psum[128 och, 512 tok] += w[k,cin,och].T @
    xT[:, k:k+512], accumulating over k in 0..3 and the group's 2 cin chunks
  - output written och-major [D, TOK] to HBM; host transposes back.
"""

import numpy as np

import concourse.bass as bass
import concourse.mybir as mybir
import concourse.tile as tile
from concourse import bacc, masks
from concourse.bass_utils import run_bass_kernel_spmd

B, L, D, K, G = 4, 4096, 2048, 4, 8
CG = D // G          # 256 channels per group
NCORES = 8
TOK = (B * L) // NCORES   # 2048 tokens per core
TT = 128                  # row tile for DMA/transpose
NCHUNK = D // 128         # 16 cin chunks of 128
PAD = K - 1               # 3 (causal left pad)
HPAD = 4                  # halo rows fetched (fp32r transpose needs even cols)

F32 = mybir.dt.float32
F32R = mybir.dt.float32r
MM_F32R = True            # conv matmuls + transposes in float32r

TB = 512                  # token block for the matmul moving dim
NB = TOK // TB            # 4 token blocks per core


def _emit(tc, nc, xs, wt, y):
    """y is [D, TOK] (transposed); host un-transposes."""
    import contextlib
    ctx = contextlib.ExitStack()
    mmdt = F32R if MM_F32R else F32
    with ctx:
        constp = ctx.enter_context(tc.tile_pool(name="constp", bufs=1))
        wp = ctx.enter_context(tc.tile_pool(name="wp", bufs=1))
        xinp = ctx.enter_context(tc.tile_pool(name="xinp", bufs=8))
        halop = ctx.enter_context(tc.tile_pool(name="halop", bufs=2))
        xtp = ctx.enter_context(tc.tile_pool(name="xtp", bufs=6))
        outp = ctx.enter_context(tc.tile_pool(name="outp", bufs=4))
        pm = ctx.enter_context(tc.tile_pool(name="pm", bufs=2, space="PSUM"))
        ph = ctx.enter_context(tc.tile_pool(name="ph", bufs=2, space="PSUM"))
        po = ctx.enter_context(tc.tile_pool(name="po", bufs=4, space="PSUM"))

        identf = constp.tile([128, 128], F32)
        masks.make_identity(nc, identf[:])
        if MM_F32R:
            ident = constp.tile([128, 128], F32R)
            nc.vector.tensor_copy(ident[:], identf[:])
        else:
            ident = identf

        # All loads/stores go on the single SP HWDGE FIFO; issue order below
        # is hand-interleaved so weights for group g arrive just before the
        # conv matmuls of group g, with the next block's x trickling in.
        xtiles = {t: {"xms": [None] * (TB // TT), "xh": None} for t in range(NB)}

        def dma_xm(t, i):
            t0 = t * TB
            xm = xinp.tile([TT, D], mmdt, name="xm")
            nc.sync.dma_start(
                xm[:], xs[HPAD + t0 + i * TT: HPAD + t0 + (i + 1) * TT, :]
            )
            xtiles[t]["xms"][i] = xm

        def dma_xh(t):
            t0 = t * TB
            xh = halop.tile([HPAD, D], mmdt, name="xh")
            nc.sync.dma_start(xh[:], xs[t0: t0 + HPAD, :])
            xtiles[t]["xh"] = xh

        wsb = {}
        for k in range(K):
            for j in range(2):
                wsb[(k, j)] = wp.tile([128, D], mmdt, name=f"w_{k}_{j}")

        def dma_w(g):
            for k in range(K):
                for j in range(2):
                    nc.sync.dma_start(
                        wsb[(k, j)][:, g * CG:(g + 1) * CG],
                        wt[k, j * 128:(j + 1) * 128, g * CG:(g + 1) * CG],
                    )

        # Block 0 x first, then the first two weight groups.
        dma_xh(0)
        for i in range(TB // TT):
            dma_xm(0, i)
        dma_w(0)
        dma_w(1)

        # DMA issue hooks: after emitting block t / conv-group g.
        hooks = {}
        for g in range(6):
            hooks.setdefault((0, g), []).append(lambda g=g: dma_w(g + 2))
        for i in range(4):
            hooks.setdefault((0, i + 1), []).append(lambda i=i: dma_xm(1, i))
        hooks.setdefault((0, 5), []).append(lambda: dma_xh(1))
        for t in (1, 2):
            for i in range(4):
                hooks.setdefault((t, i), []).append(
                    lambda t=t, i=i: dma_xm(t + 1, i)
                )
            hooks.setdefault((t, 4), []).append(lambda t=t: dma_xh(t + 1))

        for t in range(NB):
            t0 = t * TB
            xms, xh = xtiles[t]["xms"], xtiles[t]["xh"]

            def make_xt(c):
                # pmt cols = tokens [t0, t0+TB)
                pmt = pm.tile([128, TB], mmdt, name="pmt")
                for i in range(TB // TT):
                    nc.tensor.transpose(
                        pmt[:, i * TT:(i + 1) * TT],
                        xms[i][:, c * 128:(c + 1) * 128],
                        ident[:],
                    )
                # pht cols = tokens [t0-4, t0)
                pht = ph.tile([128, HPAD], mmdt, name="pht")
                nc.tensor.transpose(
                    pht[:], xh[:, c * 128:(c + 1) * 128], ident[0:HPAD, 0:HPAD]
                )
                # xt cols = tokens [t0-3, t0+TB)
                xt_t = xtp.tile([128, TB + PAD], mmdt, name="xt_t")
                nc.vector.tensor_copy(xt_t[:, PAD:], pmt[:])
                nc.vector.tensor_copy(xt_t[:, 0:PAD], pht[:, HPAD - PAD:])
                return xt_t

            for g in range(G):
                xt_pair = [make_xt(2 * g + 0), make_xt(2 * g + 1)]
                for cc in (2 * g, 2 * g + 1):
                    pot = po.tile([128, TB], F32, name="pot")
                    first = True
                    for j in range(2):
                        xt_t = xt_pair[j]
                        for k in range(K):
                            nc.tensor.matmul(
                                pot[:],
                                wsb[(k, j)][:, cc * 128:(cc + 1) * 128],
                                xt_t[:, k:k + TB],
                                start=first,
                                stop=(j == 1 and k == K - 1),
                            )
                            first = False
                    ot = outp.tile([128, TB], F32, name="ot")
                    nc.vector.tensor_copy(ot[:], pot[:])
                    nc.sync.dma_start(
                        y[cc * 128:(cc + 1) * 128, t0:t0 + TB], ot[:]
                    )
                for fn in hooks.get((t, g), []):
                    fn()


_NC_CACHE = None


def build_nc():
    global _NC_CACHE
    if _NC_CACHE is not None:
        return _NC_CACHE
    mmdt = F32R if MM_F32R else F32
    nc = bacc.Bacc(
        "TRN2", target_bir_lowering=False, debug=False, num_devices=NCORES
    )
    xs = nc.dram_tensor("xs", [TOK + HPAD, D], mmdt, kind="ExternalInput").ap()
    wt = nc.dram_tensor("wt", [K, CG, D], mmdt, kind="ExternalInput").ap()
    y = nc.dram_tensor("y", [D, TOK], F32, kind="ExternalOutput").ap()
    with tile.TileContext(nc) as tc:
        _emit(tc, nc, xs, wt, y)
    nc.compile()
    _NC_CACHE = nc
    return nc


def make_in_maps(x, w):
    """Shard full x [B, L, D] into 8 per-core inputs with causal halo rows."""
    x = np.ascontiguousarray(x, dtype=np.float32)
    w = np.ascontiguousarray(w, dtype=np.float32)
    halves = L // (NCORES // B)  # 2048
    in_maps = []
    for core in range(NCORES):
        b, h = divmod(core, NCORES // B)
        lo = h * halves
        shard = np.zeros((TOK + HPAD, D), dtype=np.float32)
        if lo == 0:
            shard[HPAD:] = x[b, lo:lo + TOK]
        else:
            shard[HPAD - min(lo, HPAD):] = x[b, lo - min(lo, HPAD):lo + TOK]
        in_maps.append({"xs": shard, "wt": w})
    return in_maps


def run(x, w, trace=False, **kw):
    nc = build_nc()
    res = run_bass_kernel_spmd(
        nc, make_in_maps(x, w), core_ids=list(range(NCORES)), trace=trace, **kw
    )
    halves = L // (NCORES // B)
    out = np.empty((B, L, D), dtype=np.float32)
    for core in range(NCORES):
        b, h = divmod(core, NCORES // B)
        out[b, h * halves:(h + 1) * halves] = res.results[core]["y"].T
    return out, res


def kernel(x, w):
    out, _ = run(x, w, trace=False)
    return out
